# revision 3
# baseline (speedup 1.0000x reference)
"""Trainium2 Bass kernel for AttLayer-style attention pooling (V3).

Computes, for x[B, T, D], W[D, A], b[A], u[A, 1]:
    uit = tanh(x @ W + b)            # [B, T, A]
    z   = uit @ u[:, 0]              # [B, T]
    e   = exp(z)
    a   = e / (sum_t e + 1e-7)
    y   = einsum('btd,bt->bd', x, a) # [B, D]

Sharding: pure data parallel over batch; 8 batches = 4 batch-PAIRS per core.

V3 architecture (vs the fp16 V1 baseline, which was ScalarE-bound at
~27.6us: tanh [128,T] + exp [128,T] per batch = 2 full ACT passes):

* x ships as fp8-e4m3 -> HBM traffic halves (12.6us/core roofline).
  mm1 runs as DoubleRow fp8 matmuls (2 contraction rows/cycle); the
  W-quantization error is compensated by a second accumulating DoubleRow
  pass with dW = e4m3(W - e4m3(W)).
* PAIR-OF-BATCHES layout: each SBUF partition holds 64 d-values of batch
  b (p<64) or b' (p>=64), so ONE exp([128, T]) serves TWO batches.  The
  pair z is built by two accumulating matmuls with zero-padded u
  stationaries (u2[:,0] = [u|0], u2[:,1] = [0|u]) -- no partition-offset
  tricks needed.  ScalarE cost per batch drops from tanh+exp (3.45us) to
  tanh + exp/2 (~2.9us).
* Pooling (sum_t x*e, 4096 MACs/lane/batch) splits across DVE
  (scalar_tensor_tensor with accum_out; fp8 in0 forces 1x) and GpSimd
  (tensor_tensor multiply into an fp16 scratch; Pool cannot run stt or
  free-axis reduces) with one fused DVE tensor_reduce [128, 3, T] ->
  [128, 3] finishing GpSimd's three slabs at 2x.
* Normalization is applied after pooling (y = y_raw * 1/(S+eps), with S
  from the exp's accum_out), per pair, on tiny [128, 4] DVE ops.
* Output leaves the device in raw slab order [128, pair, mq]; the host
  reassembles to [B, D] with a single numpy transpose (free).

Engine budget per batch (cost model): ACT 2.9us, DVE 2.8, GpSimd 2.7,
PE 2.6, DMA 1.6 -> ~23-25us/core vs 27.6 baseline.
"""

from contextlib import ExitStack

import numpy as np
import ml_dtypes

import concourse.bass as bass
import concourse.tile as tile
from concourse import mybir
from concourse.bass_utils import run_bass_kernel_spmd

N_CORES = 8
B, T, D, A = 64, 2048, 256, 128
BC = B // N_CORES  # batches per core
NPAIR = BC // 2  # batch pairs per core
TH = T // 2  # mm1 half size (PSUM ping-pong)
TC = 512  # matmul free-dim chunk (one PSUM bank)
EPS = 1e-7

F32 = mybir.dt.float32
F16 = mybir.dt.float16
F8 = mybir.dt.float8e4
TANH = mybir.ActivationFunctionType.Tanh
EXP = mybir.ActivationFunctionType.Exp
MULT = mybir.AluOpType.mult
ADD = mybir.AluOpType.add
DR = mybir.MatmulPerfMode.DoubleRow
AXX = mybir.AxisListType.X


def _split_multi_waits(nc):
    """Hoist all-but-one sem wait off restricted instructions onto no-ops.

    The walrus build in this container rejects instructions carrying more
    than one sync-wait command (CoreV3 setupSyncWait). A no-op on the same
    engine immediately before the instruction is semantically identical:
    the engine blocks on each wait in sequence.
    """
    counter = [0]

    def fresh_nop(engine, wait):
        counter[0] += 1
        n = mybir.InstNoOp(name=f"I-waitsplit-{counter[0]}", ins=[], outs=[])
        n.engine = engine
        n.sync_info = mybir.SyncInfo(on_wait=[wait], on_update=[])
        nc.register_instruction(n)
        return n

    for fn in nc.m.functions:
        for blk in fn.blocks:
            changed = False
            out = []
            for inst in blk.instructions:
                si = inst.sync_info
                if si is not None and si.on_wait and len(si.on_wait) > 1:
                    waits = list(si.on_wait)
                    for w in waits[:-1]:
                        out.append(fresh_nop(inst.engine, w))
                    si.on_wait = waits[-1:]
                    changed = True
                out.append(inst)
            if changed:
                blk.instructions = out
    return nc


def _emit_body(ctx, tc_, slab, wc2, dwc2, u2, bb, out, repeat=1):
    nc = tc_.nc

    singles = ctx.enter_context(tc_.tile_pool(name="singles", bufs=1))
    xpool = ctx.enter_context(tc_.tile_pool(name="slab", bufs=3))
    upool = ctx.enter_context(tc_.tile_pool(name="uit", bufs=4))
    epool = ctx.enter_context(tc_.tile_pool(name="e16", bufs=2))
    s0pool = ctx.enter_context(tc_.tile_pool(name="scr0", bufs=2))
    s1pool = ctx.enter_context(tc_.tile_pool(name="scr1", bufs=2))
    smpool = ctx.enter_context(tc_.tile_pool(name="small", bufs=4))
    pu_pool = ctx.enter_context(tc_.tile_pool(name="pu", bufs=2, space="PSUM"))
    zp_pool = ctx.enter_context(tc_.tile_pool(name="zp", bufs=1, space="PSUM"))

    wc2_sb = singles.tile([128, 2, 2, A], F8)
    nc.sync.dma_start(wc2_sb[:], wc2.ap())
    dwc2_sb = singles.tile([128, 2, 2, A], F8)
    nc.sync.dma_start(dwc2_sb[:], dwc2.ap())
    u2_sb = singles.tile([A, 2, 128], F16)
    nc.sync.dma_start(u2_sb[:], u2.ap())
    b_sb = singles.tile([A, 1], F32)
    nc.sync.dma_start(b_sb[:], bb.ap())
    s_all = singles.tile([128, NPAIR], F32)
    y_parts = singles.tile([128, NPAIR, 4], F32)
    yn = singles.tile([128, NPAIR, 4], F32)

    def load_slab(pr, split_first=False):
        sl = xpool.tile([128, 2, 2, T], F8, tag="slab")
        if split_first:
            nc.sync.dma_start(sl[:, 0], slab.ap()[pr, :, 0])
            nc.sync.dma_start(sl[:, 1], slab.ap()[pr, :, 1])
        else:
            nc.sync.dma_start(sl[:], slab.ap()[pr])
        return sl

    def mm1_tanh(sl, beta, uitT):
        # uitT[a, t] = tanh(sum_d W[d, a] x[t, d] + b[a]) for batch-half
        # beta (0 -> partitions 0:64, 1 -> 64:128).  DoubleRow contracts
        # the (q=0, q=1) d-64-pairs; dW pass compensates W quantization.
        base = 64 * beta
        for h in range(2):
            pu = pu_pool.tile([A, TH], F32, tag="pu")
            for tcn in range(TH // TC):
                sl_t = slice(h * TH + TC * tcn, h * TH + TC * (tcn + 1))
                sl_p = slice(TC * tcn, TC * (tcn + 1))
                for wci, w_sb in ((0, wc2_sb), (1, dwc2_sb)):
                    for m in range(2):
                        nc.tensor.matmul(
                            pu[:, sl_p],
                            w_sb[base : base + 64, m],
                            sl[base : base + 64, m, :, sl_t],
                            start=(wci == 0 and m == 0),
                            stop=(wci == 1 and m == 1),
                            perf_mode=DR,
                        )
            nc.scalar.activation(
                uitT[:, h * TH : (h + 1) * TH], pu[:], TANH, bias=b_sb[:]
            )

    def mm2(st):
        # z_pair[p, t] = z_b[t] for p<64 else z_b'[t], via two accumulating
        # matmuls with zero-padded u stationaries.
        zp = zp_pool.tile([128, T], F32, tag="zp")
        for tcn in range(T // TC):
            sl_t = slice(TC * tcn, TC * (tcn + 1))
            nc.tensor.matmul(
                zp[:, sl_t], u2_sb[:, 0], st["uit0"][:, sl_t],
                start=True, stop=False,
            )
            nc.tensor.matmul(
                zp[:, sl_t], u2_sb[:, 1], st["uit1"][:, sl_t],
                start=False, stop=True,
            )
        return zp

    def exp_pair(st, pr):
        e16 = epool.tile([128, T], F16, tag="e16")
        nc.scalar.activation(
            e16[:], st["zp"][:], EXP, accum_out=s_all[:, pr : pr + 1]
        )
        st["e16"] = e16

    def pool_pair(st, pr):
        # 4 slabs of [128, T] x*e products; slab (m=0, q=0) on DVE via stt
        # (fused accumulate), slabs (0,1), (1,0), (1,1) on GpSimd tensor_
        # tensor into one fp16 scratch, finished by one fused DVE reduce.
        sl, e16 = st["slab"], st["e16"]
        scr0 = s0pool.tile([128, T], F16, tag="scr0")
        nc.vector.scalar_tensor_tensor(
            out=scr0[:], in0=sl[:, 0, 0], scalar=1.0, in1=e16[:],
            op0=MULT, op1=MULT,
            accum_out=y_parts[:, pr, 0:1],
        )
        scr1 = s1pool.tile([128, 3, T], F16, tag="scr1")
        s4 = sl[:].rearrange("p m q t -> p (m q) t")
        for k in range(3):
            nc.gpsimd.tensor_tensor(
                out=scr1[:, k], in0=s4[:, 1 + k], in1=e16[:], op=MULT
            )
        nc.vector.tensor_reduce(
            out=y_parts[:, pr, 1:4], in_=scr1[:], axis=AXX, op=ADD
        )

    def norm_pair(pr):
        # y = y_raw / (S + eps), S replicated per partition-half already
        sc = smpool.tile([128, 1], F32, tag="sc")
        nc.vector.tensor_scalar_add(sc[:], s_all[:, pr : pr + 1], EPS)
        rr = smpool.tile([128, 1], F32, tag="rr")
        nc.vector.reciprocal(rr[:], sc[:])
        nc.vector.tensor_scalar(
            out=yn[:, pr], in0=y_parts[:, pr], scalar1=rr[:], scalar2=None,
            op0=MULT,
        )

    def one_pass():
        slabs = {0: load_slab(0, split_first=True)}

        def ensure(pr):
            if pr < NPAIR and pr not in slabs:
                slabs[pr] = load_slab(pr)

        ensure(1)
        sts = {}
        for pr in range(NPAIR):
            ensure(pr + 2)
            st = {"slab": slabs[pr]}
            st["uit0"] = upool.tile([A, T], F16, tag="uit", name="uit0")
            st["uit1"] = upool.tile([A, T], F16, tag="uit", name="uit1")
            mm1_tanh(st["slab"], 0, st["uit0"])
            mm1_tanh(st["slab"], 1, st["uit1"])
            sts[pr] = st
            # run pair pr-1's tail after pair pr's mm1/tanh so the ScalarE
            # never waits on mm2 and the PE never waits on tanh
            if pr > 0:
                tail(sts, pr - 1)
        tail(sts, NPAIR - 1)
        nc.sync.dma_start(out.ap(), yn[:])

    def tail(sts, pr):
        st = sts[pr]
        st["zp"] = mm2(st)
        exp_pair(st, pr)
        pool_pair(st, pr)
        norm_pair(pr)
        del sts[pr]

    for _ in range(repeat):
        one_pass()


_NC_CACHE = {}


def _build_nc(repeat=1, hw_loop=False):
    key = (repeat, hw_loop)
    if key in _NC_CACHE:
        return _NC_CACHE[key]
    nc = bass.Bass()
    slab = nc.declare_dram_parameter("slab", [NPAIR, 128, 2, 2, T], F8, isOutput=False)
    wc2 = nc.declare_dram_parameter("wc2", [128, 2, 2, A], F8, isOutput=False)
    dwc2 = nc.declare_dram_parameter("dwc2", [128, 2, 2, A], F8, isOutput=False)
    u2 = nc.declare_dram_parameter("u2", [A, 2, 128], F16, isOutput=False)
    bb = nc.declare_dram_parameter("bb", [A, 1], F32, isOutput=False)
    out = nc.declare_dram_parameter("out", [128, NPAIR, 4], F32, isOutput=True)
    with tile.TileContext(nc) as tc_, ExitStack() as ctx:
        _emit_body(ctx, tc_, slab, wc2, dwc2, u2, bb, out, repeat=repeat)
    _split_multi_waits(nc)
    _NC_CACHE[key] = nc
    return nc


def make_in_maps(x, W, b, u):
    x = np.asarray(x, dtype=np.float32)
    x8 = x.astype(ml_dtypes.float8_e4m3)
    # slab[pr, (h, pp), m, q, t] = x8[2pr+h, t, (2m+q)*64+pp]
    xr = x8.reshape(B // 2, 2, T, 2, 2, 64)
    slab = np.ascontiguousarray(xr.transpose(0, 1, 5, 3, 4, 2)).reshape(
        B // 2, 128, 2, 2, T
    )

    Wf = np.asarray(W, dtype=np.float32)
    W8 = Wf.astype(ml_dtypes.float8_e4m3)
    dW8 = (Wf - W8.astype(np.float32)).astype(ml_dtypes.float8_e4m3)

    def wlayout(w):
        wh = np.ascontiguousarray(
            w.reshape(2, 2, 64, A).transpose(2, 0, 1, 3)
        )
        return np.concatenate([wh, wh], axis=0)

    wc2 = wlayout(W8)
    dwc2 = wlayout(dW8)

    uf = np.asarray(u, dtype=np.float32).reshape(A)
    u2 = np.zeros((A, 2, 128), dtype=np.float16)
    u2[:, 0, 0:64] = uf[:, None].astype(np.float16)
    u2[:, 1, 64:128] = uf[:, None].astype(np.float16)
    bb = np.asarray(b, dtype=np.float32).reshape(A, 1).copy()

    return [
        {
            "slab": slab[c * NPAIR : (c + 1) * NPAIR],
            "wc2": wc2,
            "dwc2": dwc2,
            "u2": u2,
            "bb": bb,
        }
        for c in range(N_CORES)
    ]


def assemble(raw):
    """[n*128, NPAIR, 4] raw device output -> [n*BC/..., D] batch-major."""
    raw = np.asarray(raw)
    ncores = raw.shape[0] // 128
    outs = []
    for c in range(ncores):
        o = raw[c * 128 : (c + 1) * 128].reshape(2, 64, NPAIR, 4)
        outs.append(o.transpose(2, 0, 3, 1).reshape(BC, D))
    return np.concatenate(outs, axis=0)


def kernel(x, W, b, u):
    nc = _build_nc()
    res = run_bass_kernel_spmd(nc, make_in_maps(x, W, b, u), list(range(N_CORES)))
    return assemble(np.concatenate([r["out"] for r in res.results], axis=0))
